# revision 1
# baseline (speedup 1.0000x reference)
"""Histogram-equalization kernel for Trainium2 (Bass), 8-core data parallel.

Input:  images [64, 512, 512, 3] int32 (values 0..255)
Output: [64, 512, 512, 3] uint8 (per-image per-channel equalization).

Per core: 8 images = 24 channels of 262144 px, as [128, 2048] int16 tiles.

This platform charges a large fixed cost per engine instruction, so the
kernel minimizes instruction count with big fused DVE ops (no PE):

  Loop 1 (per channel): deinterleave; 256-bin counts via chunked
    is_equal-vs-iota (uint8) + segmented reduce; partition fold-tree
    (64+32 TT folds, then a strided-view reduce) -> histos[ch, 256].
  Batched LUT derivation for all 24 channels on [24, 256] tiles:
    cumsum (8 shifted adds), exact step = floor(m2/255) and
    lut = floor((csprev + half)/step) via round-cast + integer residual
    correction (the fp32->int cast rounds to nearest), step==0 identity.
  Loop 2 (per channel): lut applied as out = sum_h [hi==h] * W_h,
    W_h = sum_l T[h,l]*[lo==l], chunked; all products have exactly one
    nonzero term so bf16 stays exact. Strided uint8 write interleaves RGB.
"""

import sys

sys.path.insert(0, "/opt/trn_rl_repo")

import numpy as np

P = 128
H = W = 512
CH = 3
IMG_PER_CORE = 8
N_CORES = 8
F = (H * W) // P  # 2048
NPX = H * W
FH = 128  # histogram chunk: 256*FH = 32768 fits 16-bit ISA fields
FA = 256  # apply chunk (prod tile [128, 16*FA*16] uint8 = 64KB/part)

_cache = {}


def build(n_img=IMG_PER_CORE, debug=False):
    from contextlib import ExitStack

    import concourse.bacc as bacc
    import concourse.mybir as mybir
    from concourse.tile import TileContext

    dt = mybir.dt
    Alu = mybir.AluOpType
    AX = mybir.AxisListType

    nch = n_img * CH
    nc = bacc.Bacc("TRN2", target_bir_lowering=False, debug=False)
    imgs = nc.dram_tensor("imgs", [n_img, H * W * CH], dt.int32, kind="ExternalInput")
    out = nc.dram_tensor("out", [n_img, H * W * CH], dt.uint8, kind="ExternalOutput")
    dbg = None
    if debug:
        dbg = nc.dram_tensor("dbg", [nch, 256], dt.float32, kind="ExternalOutput")

    with TileContext(nc) as tc, ExitStack() as ctx:
        sb = ctx.enter_context(tc.tile_pool(name="sb", bufs=1))
        sbd = ctx.enter_context(tc.tile_pool(name="sbd", bufs=1))

        # constants materialized on all partitions (cm=0)
        iota256 = sb.tile([P, 256], dt.int16, tag="iota256")
        nc.gpsimd.iota(iota256[:], pattern=[[1, 256]], base=0, channel_multiplier=0)
        iotaL = sb.tile([P, 16], dt.int16, tag="iotaL")
        nc.gpsimd.iota(iotaL[:], pattern=[[1, 16]], base=0, channel_multiplier=0)
        iotaf = sbd.tile([nch, 256], dt.float32, tag="iotaf")
        ioti = sbd.tile([nch, 256], dt.int32, tag="ioti")
        nc.gpsimd.iota(ioti[:], pattern=[[1, 256]], base=0, channel_multiplier=0)
        nc.vector.tensor_copy(iotaf[:], ioti[:])

        histos = sbd.tile([nch, 256], dt.float32, tag="histos")

        # ---------- Loop 1: histograms ----------
        for img in range(n_img):
            img32 = sb.tile([P, H * W * CH // P], dt.int32, tag="img32")
            nc.sync.dma_start(out=img32[:], in_=imgs[img : img + 1, :])
            img16 = img32[:].bitcast(dt.int16)
            for c in range(CH):
                ch = img * CH + c
                x16 = sb.tile([P, F], dt.int16, tag="x16")
                nc.vector.tensor_copy(x16[:], img16[:, 2 * c :: 6])

                part = sb.tile([P, 256], dt.uint16, tag="part")
                for k in range(F // FH):
                    eq = sb.tile([P, 256 * FH], dt.uint8, tag="big")
                    # eq[p, b*FH + f] = (x16[p, k*FH + f] == b)
                    nc.vector.tensor_tensor(
                        out=eq[:],
                        in0=x16[:, k * FH : (k + 1) * FH]
                        .unsqueeze(1)
                        .to_broadcast([P, 256, FH]),
                        in1=iota256[:].unsqueeze(2).to_broadcast([P, 256, FH]),
                        op=Alu.is_equal,
                    )
                    pk = sb.tile([P, 256], dt.uint16, tag="pk")
                    with nc.allow_low_precision(
                        reason="integer counts <= 256 fit uint16 exactly"
                    ):
                        nc.vector.tensor_reduce(
                            out=pk[:],
                            in_=eq[:].rearrange("p (b f) -> p b f", f=FH),
                            axis=AX.X,
                            op=Alu.add,
                        )
                    if k == 0:
                        nc.vector.tensor_copy(part[:], pk[:])
                    else:
                        nc.vector.tensor_tensor(
                            out=part[:], in0=part[:], in1=pk[:], op=Alu.add
                        )
                # gather all 128 rows into one row, reduce with strided view
                row128 = sb.tile([1, P * 256], dt.uint16, tag="row128")
                nc.sync.dma_start(out=row128[:], in_=part[:])
                # row128[0, p*256 + b]; reduce over p via [1, 256(b), 128(p)]
                hrow = sb.tile([1, 256], dt.float32, tag="hrow")
                nc.vector.tensor_reduce(
                    out=hrow[:],
                    in_=row128[:].rearrange("o (pp b) -> o b pp", b=256),
                    axis=AX.X,
                    op=Alu.add,
                )
                nc.sync.dma_start(out=histos[ch : ch + 1, :], in_=hrow[:])

        # ---------- Batched LUT derivation on [nch, 256] ----------
        NC2 = nch
        ca = sbd.tile([NC2, 256], dt.float32, tag="ca")
        cb = sbd.tile([NC2, 256], dt.float32, tag="cb")
        src = histos
        for k in range(8):
            s = 1 << k
            dst = ca if (k % 2 == 0) else cb
            nc.vector.tensor_copy(dst[:, :s], src[:, :s])
            nc.vector.tensor_tensor(
                out=dst[:, s:256], in0=src[:, s:256], in1=src[:, : 256 - s],
                op=Alu.add,
            )
            src = dst
        cum = src  # cb
        t1 = ca

        nc.vector.tensor_scalar(
            out=t1[:], in0=cum[:], scalar1=float(NPX), scalar2=None, op0=Alu.is_lt
        )
        nc.vector.tensor_tensor(out=t1[:], in0=t1[:], in1=cum[:], op=Alu.mult)
        m2 = sbd.tile([NC2, 1], dt.float32, tag="m2")
        nc.vector.tensor_reduce(out=m2[:], in_=t1[:], axis=AX.X, op=Alu.max)

        stepf = sbd.tile([NC2, 1], dt.float32, tag="stepf")
        nc.vector.tensor_scalar(
            out=stepf[:], in0=m2[:], scalar1=1.0 / 255.0, scalar2=None, op0=Alu.mult
        )
        stepi = sbd.tile([NC2, 1], dt.int32, tag="stepi")
        nc.vector.tensor_copy(stepi[:], stepf[:])
        nc.vector.tensor_copy(stepf[:], stepi[:])
        se = sbd.tile([NC2, 1], dt.float32, tag="se")
        nc.vector.tensor_scalar(
            out=se[:], in0=stepf[:], scalar1=-255.0, scalar2=None, op0=Alu.mult
        )
        nc.vector.tensor_tensor(out=se[:], in0=m2[:], in1=se[:], op=Alu.add)
        scor = sbd.tile([NC2, 1], dt.float32, tag="scor")
        nc.vector.tensor_scalar(
            out=scor[:], in0=se[:], scalar1=0.0, scalar2=None, op0=Alu.is_lt
        )
        nc.vector.tensor_tensor(
            out=stepf[:], in0=stepf[:], in1=scor[:], op=Alu.subtract
        )
        nc.vector.tensor_scalar(
            out=scor[:], in0=se[:], scalar1=255.0, scalar2=None, op0=Alu.is_ge
        )
        nc.vector.tensor_tensor(out=stepf[:], in0=stepf[:], in1=scor[:], op=Alu.add)

        s_f = sbd.tile([NC2, 1], dt.float32, tag="s_f")
        nc.vector.tensor_scalar(
            out=s_f[:], in0=stepf[:], scalar1=1.0, scalar2=None, op0=Alu.max
        )
        halff = sbd.tile([NC2, 1], dt.float32, tag="halff")
        halfi = sbd.tile([NC2, 1], dt.int32, tag="halfi")
        nc.vector.tensor_scalar(
            out=halff[:], in0=s_f[:], scalar1=0.5, scalar2=-0.25,
            op0=Alu.mult, op1=Alu.add,
        )
        nc.vector.tensor_copy(halfi[:], halff[:])
        nc.vector.tensor_copy(halff[:], halfi[:])

        r0 = sbd.tile([NC2, 1], dt.float32, tag="r0")
        nc.vector.reciprocal(r0[:], s_f[:])
        tn = sbd.tile([NC2, 1], dt.float32, tag="tn")
        nc.vector.tensor_tensor(out=tn[:], in0=s_f[:], in1=r0[:], op=Alu.mult)
        nc.vector.tensor_scalar(
            out=tn[:], in0=tn[:], scalar1=-1.0, scalar2=2.0, op0=Alu.mult, op1=Alu.add
        )
        r1 = sbd.tile([NC2, 1], dt.float32, tag="r1")
        nc.vector.tensor_tensor(out=r1[:], in0=r0[:], in1=tn[:], op=Alu.mult)

        csp = sbd.tile([NC2, 256], dt.float32, tag="csp")
        nc.vector.memset(csp[:, :1], 0.0)
        nc.vector.tensor_copy(csp[:, 1:256], cum[:, :255])

        num = sbd.tile([NC2, 256], dt.float32, tag="num")
        nc.vector.tensor_scalar(
            out=num[:], in0=csp[:], scalar1=halff[:, :1], scalar2=r1[:, :1],
            op0=Alu.add, op1=Alu.mult,
        )
        q0i = sbd.tile([NC2, 256], dt.int32, tag="q0i")
        nc.vector.tensor_copy(q0i[:], num[:])
        q0 = sbd.tile([NC2, 256], dt.float32, tag="q0")
        nc.vector.tensor_copy(q0[:], q0i[:])

        e = sbd.tile([NC2, 256], dt.float32, tag="e")
        nc.vector.tensor_scalar(
            out=e[:], in0=q0[:], scalar1=s_f[:, :1], scalar2=None, op0=Alu.mult
        )
        nc.vector.tensor_tensor(out=e[:], in0=csp[:], in1=e[:], op=Alu.subtract)
        nc.vector.tensor_scalar(
            out=e[:], in0=e[:], scalar1=halff[:, :1], scalar2=None, op0=Alu.add
        )
        corr = sbd.tile([NC2, 256], dt.float32, tag="corr")
        nc.vector.tensor_scalar(
            out=corr[:], in0=e[:], scalar1=s_f[:, :1], scalar2=None, op0=Alu.is_ge
        )
        nc.vector.tensor_tensor(out=q0[:], in0=q0[:], in1=corr[:], op=Alu.add)
        nc.vector.tensor_scalar(
            out=corr[:], in0=e[:], scalar1=0.0, scalar2=None, op0=Alu.is_lt
        )
        nc.vector.tensor_tensor(out=q0[:], in0=q0[:], in1=corr[:], op=Alu.subtract)
        nc.vector.tensor_scalar(
            out=q0[:], in0=q0[:], scalar1=0.0, scalar2=255.0, op0=Alu.max, op1=Alu.min
        )

        m0 = sbd.tile([NC2, 1], dt.float32, tag="m0")
        nc.vector.tensor_scalar(
            out=m0[:], in0=stepf[:], scalar1=0.0, scalar2=None, op0=Alu.is_equal
        )
        lut = sbd.tile([NC2, 256], dt.float32, tag="lut")
        nc.vector.tensor_tensor(out=lut[:], in0=iotaf[:], in1=q0[:], op=Alu.subtract)
        nc.vector.tensor_scalar(
            out=lut[:], in0=lut[:], scalar1=m0[:, :1], scalar2=None, op0=Alu.mult
        )
        nc.vector.tensor_tensor(out=lut[:], in0=lut[:], in1=q0[:], op=Alu.add)
        lutb = sbd.tile([NC2, 256], dt.uint8, tag="lutb")
        nc.vector.tensor_copy(lutb[:], lut[:])
        if debug:
            nc.sync.dma_start(out=dbg[:, :], in_=lut[:])

        # ---------- Loop 2: apply ----------
        for img in range(n_img):
            img32b = sb.tile([P, H * W * CH // P], dt.int32, tag="img32")
            nc.sync.dma_start(out=img32b[:], in_=imgs[img : img + 1, :])
            img16b = img32b[:].bitcast(dt.int16)
            org = sb.tile([P, CH * F], dt.uint8, tag="org")
            for c in range(CH):
                ch = img * CH + c
                x16 = sb.tile([P, F], dt.int16, tag="x16")
                nc.vector.tensor_copy(x16[:], img16b[:, 2 * c :: 6])
                lo16 = sb.tile([P, F], dt.int16, tag="lo16")
                hi16 = sb.tile([P, F], dt.int16, tag="hi16")
                nc.vector.tensor_scalar(
                    out=lo16[:], in0=x16[:], scalar1=15, scalar2=None,
                    op0=Alu.bitwise_and,
                )
                nc.vector.tensor_scalar(
                    out=hi16[:], in0=x16[:], scalar1=4, scalar2=None,
                    op0=Alu.logical_shift_right,
                )
                # replicate this channel's lut row to all partitions, bf16
                T128 = sb.tile([P, 256], dt.uint8, tag="T128")
                nc.sync.dma_start(
                    out=T128[:],
                    in_=lutb[ch : ch + 1, :].unsqueeze(1).to_broadcast([1, P, 256]),
                )
                outb = sb.tile([P, F], dt.uint8, tag="outb")
                for k in range(F // FA):
                    sl = slice(k * FA, (k + 1) * FA)
                    # slabL chunk [P, 16l * FA] (l-major)
                    slabLc = sb.tile([P, 16 * FA], dt.uint8, tag="slabLc")
                    nc.vector.tensor_tensor(
                        out=slabLc[:],
                        in0=lo16[:, sl].unsqueeze(1).to_broadcast([P, 16, FA]),
                        in1=iotaL[:].unsqueeze(2).to_broadcast([P, 16, FA]),
                        op=Alu.is_equal,
                    )
                    slabHc = sb.tile([P, 16 * FA], dt.uint8, tag="slabHc")
                    nc.vector.tensor_tensor(
                        out=slabHc[:],
                        in0=hi16[:, sl].unsqueeze(1).to_broadcast([P, 16, FA]),
                        in1=iotaL[:].unsqueeze(2).to_broadcast([P, 16, FA]),
                        op=Alu.is_equal,
                    )
                    # prod[p, (h, f, l)] = slabLc[p, l*FA + f] * T128[p, 16h + l]
                    prod = sb.tile([P, 16 * FA * 16], dt.uint8, tag="big")
                    half = 8 * FA * 16
                    for hh in range(2):
                        nc.vector.tensor_tensor(
                            out=prod[:, hh * half : (hh + 1) * half],
                            in0=slabLc[:]
                            .rearrange("p (l f) -> p f l", l=16)
                            .unsqueeze(1)
                            .to_broadcast([P, 8, FA, 16]),
                            in1=T128[:, hh * 128 : (hh + 1) * 128]
                            .rearrange("p (h l) -> p h l", l=16)
                            .unsqueeze(2)
                            .to_broadcast([P, 8, FA, 16]),
                            op=Alu.mult,
                        )
                    # W[p, (h, f)] = sum_l prod
                    Wc = sb.tile([P, 16 * FA], dt.uint8, tag="Wc")
                    with nc.allow_low_precision(
                        reason="sums have exactly one nonzero bf16 term"
                    ):
                        nc.vector.tensor_reduce(
                        out=Wc[:],
                            in_=prod[:].rearrange(
                                "p (h f l) -> p (h f) l", l=16, f=FA
                            ),
                            axis=AX.X,
                            op=Alu.add,
                        )
                    # prod2[p, (f, h)] = slabHc * Wc (both (h, f) viewed as (f, h))
                    prod2 = sb.tile([P, FA * 16], dt.uint8, tag="prod2")
                    nc.vector.tensor_tensor(
                        out=prod2[:],
                        in0=slabHc[:].rearrange("p (h f) -> p f h", h=16),
                        in1=Wc[:].rearrange("p (h f) -> p f h", h=16),
                        op=Alu.mult,
                    )
                    with nc.allow_low_precision(
                        reason="sums have exactly one nonzero bf16 term"
                    ):
                        nc.vector.tensor_reduce(
                            out=outb[:, sl],
                            in_=prod2[:].rearrange("p (f h) -> p f h", h=16),
                            axis=AX.X,
                            op=Alu.add,
                        )
                # interleave into RGB layout (strided uint8 write)
                nc.vector.tensor_copy(org[:, c :: CH], outb[:])
            nc.sync.dma_start(out=out[img : img + 1, :], in_=org[:])

    nc.compile()
    return nc


def numpy_ref_channel(img_ch):
    flat = np.asarray(img_ch).reshape(-1)
    histo = np.bincount(flat, minlength=256)
    nz = np.nonzero(histo)[0]
    last_nonzero = histo[nz[-1]] if len(nz) else 0
    step = (histo.sum() - last_nonzero) // 255
    safe_step = max(step, 1)
    lut = (np.cumsum(histo) + safe_step // 2) // safe_step
    lut = np.concatenate([[0], lut[:-1]])
    lut = np.clip(lut, 0, 255)
    if step == 0:
        return flat.reshape(img_ch.shape).astype(np.uint8)
    return lut[flat].reshape(img_ch.shape).astype(np.uint8)


def kernel(images: np.ndarray) -> np.ndarray:
    from concourse.bass_utils import run_bass_kernel_spmd

    if "nc" not in _cache:
        _cache["nc"] = build()
    nc = _cache["nc"]

    B = images.shape[0]
    flat = np.ascontiguousarray(images.reshape(B, -1).astype(np.int32))
    per = B // N_CORES
    in_maps = [{"imgs": flat[i * per : (i + 1) * per]} for i in range(N_CORES)]
    res = run_bass_kernel_spmd(nc, in_maps, core_ids=list(range(N_CORES)))
    outs = [r["out"] for r in res.results]
    return np.concatenate(outs, axis=0).reshape(B, H, W, CH).astype(np.uint8)



# revision 2
# speedup vs baseline: 7.5641x; 7.5641x over previous
"""Histogram-equalization kernel for Trainium2 (Bass), 8-core data parallel.

Input:  images [64, 512, 512, 3] int32 (values 0..255)
Output: [64, 512, 512, 3] uint8 (per-image per-channel equalization).

Wall-clock here is dominated by host<->device transfer and dispatch, so the
host path is organized around minimizing bytes moved and per-call overhead:

  - pixels are cast to uint8 on host (4x fewer upload bytes than int32);
  - the Bass program is compiled once and wrapped in a single cached
    jax.jit(shard_map(bass_exec)) callable (run_bass_kernel_spmd re-traces
    and re-lowers on every call, and ships 50MB of zero-filled output
    buffers per call on top of the input);
  - the batch is processed in CHUNK-image slices, dispatched
    asynchronously so host casting, uploads, device exec and downloads
    pipeline against each other.

Device side per core: n_img images, 3 channels of 262144 px each as
[128, 2048] int16 tiles.  Histogram via chunked is_equal-vs-iota + segmented
reduce; LUT derivation batched on [nch, 256] tiles (exact integer math via
round-cast + residual correction); LUT applied as a 16x16 (hi/lo nibble)
one-hot product chain; strided uint8 write re-interleaves RGB.
"""

import sys

sys.path.insert(0, "/opt/trn_rl_repo")

import numpy as np

P = 128
H = W = 512
CH = 3
N_CORES = 8
CHUNK = 16  # images per dispatch (CHUNK // N_CORES per core)
F = (H * W) // P  # 2048
NPX = H * W
FH = 128  # histogram chunk: 256*FH = 32768 fits 16-bit ISA fields
FA = 256  # apply chunk (prod tile [128, 16*FA*16] uint8 = 64KB/part)

_cache = {}


def build(n_img, debug=False):
    from contextlib import ExitStack

    import concourse.bacc as bacc
    import concourse.mybir as mybir
    from concourse.tile import TileContext

    dt = mybir.dt
    Alu = mybir.AluOpType
    AX = mybir.AxisListType

    nch = n_img * CH
    nc = bacc.Bacc("TRN2", target_bir_lowering=False, debug=False)
    imgs = nc.dram_tensor("imgs", [n_img, H * W * CH], dt.uint8, kind="ExternalInput")
    out = nc.dram_tensor("out", [n_img, H * W * CH], dt.uint8, kind="ExternalOutput")
    dbg = None
    if debug:
        dbg = nc.dram_tensor("dbg", [nch, 256], dt.float32, kind="ExternalOutput")

    with TileContext(nc) as tc, ExitStack() as ctx:
        sb = ctx.enter_context(tc.tile_pool(name="sb", bufs=1))
        sbd = ctx.enter_context(tc.tile_pool(name="sbd", bufs=1))

        # constants materialized on all partitions (cm=0)
        iota256 = sb.tile([P, 256], dt.int16, tag="iota256")
        nc.gpsimd.iota(iota256[:], pattern=[[1, 256]], base=0, channel_multiplier=0)
        iotaL = sb.tile([P, 16], dt.int16, tag="iotaL")
        nc.gpsimd.iota(iotaL[:], pattern=[[1, 16]], base=0, channel_multiplier=0)
        iotaf = sbd.tile([nch, 256], dt.float32, tag="iotaf")
        ioti = sbd.tile([nch, 256], dt.int32, tag="ioti")
        nc.gpsimd.iota(ioti[:], pattern=[[1, 256]], base=0, channel_multiplier=0)
        nc.vector.tensor_copy(iotaf[:], ioti[:])

        histos = sbd.tile([nch, 256], dt.float32, tag="histos")

        # ---------- Loop 1: histograms ----------
        for img in range(n_img):
            img8 = sb.tile([P, H * W * CH // P], dt.uint8, tag="img8")
            nc.sync.dma_start(out=img8[:], in_=imgs[img : img + 1, :])
            for c in range(CH):
                ch = img * CH + c
                x16 = sb.tile([P, F], dt.int16, tag="x16")
                nc.vector.tensor_copy(x16[:], img8[:, c::3])

                part = sb.tile([P, 256], dt.uint16, tag="part")
                for k in range(F // FH):
                    eq = sb.tile([P, 256 * FH], dt.uint8, tag="big")
                    # eq[p, b*FH + f] = (x16[p, k*FH + f] == b)
                    nc.vector.tensor_tensor(
                        out=eq[:],
                        in0=x16[:, k * FH : (k + 1) * FH]
                        .unsqueeze(1)
                        .to_broadcast([P, 256, FH]),
                        in1=iota256[:].unsqueeze(2).to_broadcast([P, 256, FH]),
                        op=Alu.is_equal,
                    )
                    pk = sb.tile([P, 256], dt.uint16, tag="pk")
                    with nc.allow_low_precision(
                        reason="integer counts <= 256 fit uint16 exactly"
                    ):
                        nc.vector.tensor_reduce(
                            out=pk[:],
                            in_=eq[:].rearrange("p (b f) -> p b f", f=FH),
                            axis=AX.X,
                            op=Alu.add,
                        )
                    if k == 0:
                        nc.vector.tensor_copy(part[:], pk[:])
                    else:
                        nc.vector.tensor_tensor(
                            out=part[:], in0=part[:], in1=pk[:], op=Alu.add
                        )
                # gather all 128 rows into one row, reduce with strided view
                row128 = sb.tile([1, P * 256], dt.uint16, tag="row128")
                nc.sync.dma_start(out=row128[:], in_=part[:])
                # row128[0, p*256 + b]; reduce over p via [1, 256(b), 128(p)]
                hrow = sb.tile([1, 256], dt.float32, tag="hrow")
                nc.vector.tensor_reduce(
                    out=hrow[:],
                    in_=row128[:].rearrange("o (pp b) -> o b pp", b=256),
                    axis=AX.X,
                    op=Alu.add,
                )
                nc.sync.dma_start(out=histos[ch : ch + 1, :], in_=hrow[:])

        # ---------- Batched LUT derivation on [nch, 256] ----------
        NC2 = nch
        ca = sbd.tile([NC2, 256], dt.float32, tag="ca")
        cb = sbd.tile([NC2, 256], dt.float32, tag="cb")
        src = histos
        for k in range(8):
            s = 1 << k
            dst = ca if (k % 2 == 0) else cb
            nc.vector.tensor_copy(dst[:, :s], src[:, :s])
            nc.vector.tensor_tensor(
                out=dst[:, s:256], in0=src[:, s:256], in1=src[:, : 256 - s],
                op=Alu.add,
            )
            src = dst
        cum = src  # cb
        t1 = ca

        nc.vector.tensor_scalar(
            out=t1[:], in0=cum[:], scalar1=float(NPX), scalar2=None, op0=Alu.is_lt
        )
        nc.vector.tensor_tensor(out=t1[:], in0=t1[:], in1=cum[:], op=Alu.mult)
        m2 = sbd.tile([NC2, 1], dt.float32, tag="m2")
        nc.vector.tensor_reduce(out=m2[:], in_=t1[:], axis=AX.X, op=Alu.max)

        stepf = sbd.tile([NC2, 1], dt.float32, tag="stepf")
        nc.vector.tensor_scalar(
            out=stepf[:], in0=m2[:], scalar1=1.0 / 255.0, scalar2=None, op0=Alu.mult
        )
        stepi = sbd.tile([NC2, 1], dt.int32, tag="stepi")
        nc.vector.tensor_copy(stepi[:], stepf[:])
        nc.vector.tensor_copy(stepf[:], stepi[:])
        se = sbd.tile([NC2, 1], dt.float32, tag="se")
        nc.vector.tensor_scalar(
            out=se[:], in0=stepf[:], scalar1=-255.0, scalar2=None, op0=Alu.mult
        )
        nc.vector.tensor_tensor(out=se[:], in0=m2[:], in1=se[:], op=Alu.add)
        scor = sbd.tile([NC2, 1], dt.float32, tag="scor")
        nc.vector.tensor_scalar(
            out=scor[:], in0=se[:], scalar1=0.0, scalar2=None, op0=Alu.is_lt
        )
        nc.vector.tensor_tensor(
            out=stepf[:], in0=stepf[:], in1=scor[:], op=Alu.subtract
        )
        nc.vector.tensor_scalar(
            out=scor[:], in0=se[:], scalar1=255.0, scalar2=None, op0=Alu.is_ge
        )
        nc.vector.tensor_tensor(out=stepf[:], in0=stepf[:], in1=scor[:], op=Alu.add)

        s_f = sbd.tile([NC2, 1], dt.float32, tag="s_f")
        nc.vector.tensor_scalar(
            out=s_f[:], in0=stepf[:], scalar1=1.0, scalar2=None, op0=Alu.max
        )
        halff = sbd.tile([NC2, 1], dt.float32, tag="halff")
        halfi = sbd.tile([NC2, 1], dt.int32, tag="halfi")
        nc.vector.tensor_scalar(
            out=halff[:], in0=s_f[:], scalar1=0.5, scalar2=-0.25,
            op0=Alu.mult, op1=Alu.add,
        )
        nc.vector.tensor_copy(halfi[:], halff[:])
        nc.vector.tensor_copy(halff[:], halfi[:])

        r0 = sbd.tile([NC2, 1], dt.float32, tag="r0")
        nc.vector.reciprocal(r0[:], s_f[:])
        tn = sbd.tile([NC2, 1], dt.float32, tag="tn")
        nc.vector.tensor_tensor(out=tn[:], in0=s_f[:], in1=r0[:], op=Alu.mult)
        nc.vector.tensor_scalar(
            out=tn[:], in0=tn[:], scalar1=-1.0, scalar2=2.0, op0=Alu.mult, op1=Alu.add
        )
        r1 = sbd.tile([NC2, 1], dt.float32, tag="r1")
        nc.vector.tensor_tensor(out=r1[:], in0=r0[:], in1=tn[:], op=Alu.mult)

        csp = sbd.tile([NC2, 256], dt.float32, tag="csp")
        nc.vector.memset(csp[:, :1], 0.0)
        nc.vector.tensor_copy(csp[:, 1:256], cum[:, :255])

        num = sbd.tile([NC2, 256], dt.float32, tag="num")
        nc.vector.tensor_scalar(
            out=num[:], in0=csp[:], scalar1=halff[:, :1], scalar2=r1[:, :1],
            op0=Alu.add, op1=Alu.mult,
        )
        q0i = sbd.tile([NC2, 256], dt.int32, tag="q0i")
        nc.vector.tensor_copy(q0i[:], num[:])
        q0 = sbd.tile([NC2, 256], dt.float32, tag="q0")
        nc.vector.tensor_copy(q0[:], q0i[:])

        e = sbd.tile([NC2, 256], dt.float32, tag="e")
        nc.vector.tensor_scalar(
            out=e[:], in0=q0[:], scalar1=s_f[:, :1], scalar2=None, op0=Alu.mult
        )
        nc.vector.tensor_tensor(out=e[:], in0=csp[:], in1=e[:], op=Alu.subtract)
        nc.vector.tensor_scalar(
            out=e[:], in0=e[:], scalar1=halff[:, :1], scalar2=None, op0=Alu.add
        )
        corr = sbd.tile([NC2, 256], dt.float32, tag="corr")
        nc.vector.tensor_scalar(
            out=corr[:], in0=e[:], scalar1=s_f[:, :1], scalar2=None, op0=Alu.is_ge
        )
        nc.vector.tensor_tensor(out=q0[:], in0=q0[:], in1=corr[:], op=Alu.add)
        nc.vector.tensor_scalar(
            out=corr[:], in0=e[:], scalar1=0.0, scalar2=None, op0=Alu.is_lt
        )
        nc.vector.tensor_tensor(out=q0[:], in0=q0[:], in1=corr[:], op=Alu.subtract)
        nc.vector.tensor_scalar(
            out=q0[:], in0=q0[:], scalar1=0.0, scalar2=255.0, op0=Alu.max, op1=Alu.min
        )

        m0 = sbd.tile([NC2, 1], dt.float32, tag="m0")
        nc.vector.tensor_scalar(
            out=m0[:], in0=stepf[:], scalar1=0.0, scalar2=None, op0=Alu.is_equal
        )
        lut = sbd.tile([NC2, 256], dt.float32, tag="lut")
        nc.vector.tensor_tensor(out=lut[:], in0=iotaf[:], in1=q0[:], op=Alu.subtract)
        nc.vector.tensor_scalar(
            out=lut[:], in0=lut[:], scalar1=m0[:, :1], scalar2=None, op0=Alu.mult
        )
        nc.vector.tensor_tensor(out=lut[:], in0=lut[:], in1=q0[:], op=Alu.add)
        lutb = sbd.tile([NC2, 256], dt.uint8, tag="lutb")
        nc.vector.tensor_copy(lutb[:], lut[:])
        if debug:
            nc.sync.dma_start(out=dbg[:, :], in_=lut[:])

        # ---------- Loop 2: apply ----------
        for img in range(n_img):
            img8b = sb.tile([P, H * W * CH // P], dt.uint8, tag="img8")
            nc.sync.dma_start(out=img8b[:], in_=imgs[img : img + 1, :])
            org = sb.tile([P, CH * F], dt.uint8, tag="org")
            for c in range(CH):
                ch = img * CH + c
                x16 = sb.tile([P, F], dt.int16, tag="x16")
                nc.vector.tensor_copy(x16[:], img8b[:, c::3])
                lo16 = sb.tile([P, F], dt.int16, tag="lo16")
                hi16 = sb.tile([P, F], dt.int16, tag="hi16")
                nc.vector.tensor_scalar(
                    out=lo16[:], in0=x16[:], scalar1=15, scalar2=None,
                    op0=Alu.bitwise_and,
                )
                nc.vector.tensor_scalar(
                    out=hi16[:], in0=x16[:], scalar1=4, scalar2=None,
                    op0=Alu.logical_shift_right,
                )
                # replicate this channel's lut row to all partitions, bf16
                T128 = sb.tile([P, 256], dt.uint8, tag="T128")
                nc.sync.dma_start(
                    out=T128[:],
                    in_=lutb[ch : ch + 1, :].unsqueeze(1).to_broadcast([1, P, 256]),
                )
                outb = sb.tile([P, F], dt.uint8, tag="outb")
                for k in range(F // FA):
                    sl = slice(k * FA, (k + 1) * FA)
                    # slabL chunk [P, 16l * FA] (l-major)
                    slabLc = sb.tile([P, 16 * FA], dt.uint8, tag="slabLc")
                    nc.vector.tensor_tensor(
                        out=slabLc[:],
                        in0=lo16[:, sl].unsqueeze(1).to_broadcast([P, 16, FA]),
                        in1=iotaL[:].unsqueeze(2).to_broadcast([P, 16, FA]),
                        op=Alu.is_equal,
                    )
                    slabHc = sb.tile([P, 16 * FA], dt.uint8, tag="slabHc")
                    nc.vector.tensor_tensor(
                        out=slabHc[:],
                        in0=hi16[:, sl].unsqueeze(1).to_broadcast([P, 16, FA]),
                        in1=iotaL[:].unsqueeze(2).to_broadcast([P, 16, FA]),
                        op=Alu.is_equal,
                    )
                    # prod[p, (h, f, l)] = slabLc[p, l*FA + f] * T128[p, 16h + l]
                    prod = sb.tile([P, 16 * FA * 16], dt.uint8, tag="big")
                    half = 8 * FA * 16
                    for hh in range(2):
                        nc.vector.tensor_tensor(
                            out=prod[:, hh * half : (hh + 1) * half],
                            in0=slabLc[:]
                            .rearrange("p (l f) -> p f l", l=16)
                            .unsqueeze(1)
                            .to_broadcast([P, 8, FA, 16]),
                            in1=T128[:, hh * 128 : (hh + 1) * 128]
                            .rearrange("p (h l) -> p h l", l=16)
                            .unsqueeze(2)
                            .to_broadcast([P, 8, FA, 16]),
                            op=Alu.mult,
                        )
                    # W[p, (h, f)] = sum_l prod
                    Wc = sb.tile([P, 16 * FA], dt.uint8, tag="Wc")
                    with nc.allow_low_precision(
                        reason="sums have exactly one nonzero bf16 term"
                    ):
                        nc.vector.tensor_reduce(
                        out=Wc[:],
                            in_=prod[:].rearrange(
                                "p (h f l) -> p (h f) l", l=16, f=FA
                            ),
                            axis=AX.X,
                            op=Alu.add,
                        )
                    # prod2[p, (f, h)] = slabHc * Wc (both (h, f) viewed as (f, h))
                    prod2 = sb.tile([P, FA * 16], dt.uint8, tag="prod2")
                    nc.vector.tensor_tensor(
                        out=prod2[:],
                        in0=slabHc[:].rearrange("p (h f) -> p f h", h=16),
                        in1=Wc[:].rearrange("p (h f) -> p f h", h=16),
                        op=Alu.mult,
                    )
                    with nc.allow_low_precision(
                        reason="sums have exactly one nonzero bf16 term"
                    ):
                        nc.vector.tensor_reduce(
                            out=outb[:, sl],
                            in_=prod2[:].rearrange("p (f h) -> p f h", h=16),
                            axis=AX.X,
                            op=Alu.add,
                        )
                # interleave into RGB layout (strided uint8 write)
                nc.vector.tensor_copy(org[:, c :: CH], outb[:])
            nc.sync.dma_start(out=out[img : img + 1, :], in_=org[:])

    nc.compile()
    return nc


def _get_fn():
    """Build the Bass program once and wrap it in a cached
    jax.jit(shard_map(bass_exec)) callable (the same lowering path
    run_bass_kernel_spmd takes under axon, minus its per-call re-trace,
    re-lower and zero-output upload)."""
    if "fn" in _cache:
        return _cache["fn"]

    import jax
    from jax.experimental.shard_map import shard_map
    from jax.sharding import Mesh, NamedSharding, PartitionSpec
    from concourse.bass2jax import (
        _bass_exec_p,
        install_neuronx_cc_hook,
        partition_id_tensor,
    )

    install_neuronx_cc_hook()

    n_img = CHUNK // N_CORES
    nc = build(n_img)
    out_avals = (jax.core.ShapedArray((n_img, H * W * CH), np.uint8),)

    def _body(imgs):
        outs = _bass_exec_p.bind(
            imgs,
            partition_id_tensor(),
            out_avals=out_avals,
            in_names=("imgs", "partition_id"),
            out_names=("out",),
            lowering_input_output_aliases=(),
            sim_require_finite=True,
            sim_require_nnan=True,
            nc=nc,
        )
        return outs[0]

    devices = jax.devices()[:N_CORES]
    mesh = Mesh(np.asarray(devices), ("core",))
    sharded = jax.jit(
        shard_map(
            _body,
            mesh=mesh,
            in_specs=(PartitionSpec("core"),),
            out_specs=PartitionSpec("core"),
            check_rep=False,
        )
    )
    sharding = NamedSharding(mesh, PartitionSpec("core"))
    _cache["fn"] = (sharded, sharding)
    return _cache["fn"]


def kernel(images: np.ndarray) -> np.ndarray:
    import jax

    fn, sharding = _get_fn()
    B = images.shape[0]

    futs = []
    for s in range(0, B, CHUNK):
        u8 = images[s : s + CHUNK].astype(np.uint8).reshape(CHUNK, -1)
        d = jax.device_put(u8, sharding)
        r = fn(d)
        try:
            r.copy_to_host_async()
        except Exception:
            pass
        futs.append(r)

    out = np.empty((B, H * W * CH), np.uint8)
    for i, r in enumerate(futs):
        out[i * CHUNK : (i + 1) * CHUNK] = np.asarray(r)
    return out.reshape(B, H, W, CH)


# revision 3
# speedup vs baseline: 7.6261x; 1.0082x over previous
"""Histogram-equalization kernel for Trainium2 (Bass), 8-core data parallel.

Input:  images [64, 512, 512, 3] int32 (values 0..255)
Output: [64, 512, 512, 3] uint8 (per-image per-channel equalization).

Wall-clock here is dominated by host<->device transfer and dispatch, so the
host path is organized around minimizing bytes moved and per-call overhead:

  - pixels are cast to uint8 on host (4x fewer upload bytes than int32);
  - the Bass program is compiled once and wrapped in a single cached
    jax.jit(shard_map(bass_exec)) callable (run_bass_kernel_spmd re-traces
    and re-lowers on every call, and ships 50MB of zero-filled output
    buffers per call on top of the input);
  - the batch is processed in CHUNK-image slices, dispatched
    asynchronously so host casting, uploads, device exec and downloads
    pipeline against each other.

Device side per core: n_img images, 3 channels of 262144 px each as
[128, 2048] int16 tiles.  Histogram via chunked is_equal-vs-iota + segmented
reduce; LUT derivation batched on [nch, 256] tiles (exact integer math via
round-cast + residual correction); LUT applied as a 16x16 (hi/lo nibble)
one-hot product chain; strided uint8 write re-interleaves RGB.
"""

import sys

sys.path.insert(0, "/opt/trn_rl_repo")

import numpy as np

P = 128
H = W = 512
CH = 3
N_CORES = 8
CHUNK = 8  # images per dispatch (CHUNK // N_CORES per core)
F = (H * W) // P  # 2048
NPX = H * W
FH = 128  # histogram chunk: 256*FH = 32768 fits 16-bit ISA fields
FA = 256  # apply chunk (prod tile [128, 16*FA*16] uint8 = 64KB/part)

_cache = {}


def build(n_img, debug=False):
    from contextlib import ExitStack

    import concourse.bacc as bacc
    import concourse.mybir as mybir
    from concourse.tile import TileContext

    dt = mybir.dt
    Alu = mybir.AluOpType
    AX = mybir.AxisListType

    nch = n_img * CH
    nc = bacc.Bacc("TRN2", target_bir_lowering=False, debug=False)
    imgs = nc.dram_tensor("imgs", [n_img, H * W * CH], dt.uint8, kind="ExternalInput")
    out = nc.dram_tensor("out", [n_img, H * W * CH], dt.uint8, kind="ExternalOutput")
    dbg = None
    if debug:
        dbg = nc.dram_tensor("dbg", [nch, 256], dt.float32, kind="ExternalOutput")

    with TileContext(nc) as tc, ExitStack() as ctx:
        sb = ctx.enter_context(tc.tile_pool(name="sb", bufs=1))
        sbd = ctx.enter_context(tc.tile_pool(name="sbd", bufs=1))

        # constants materialized on all partitions (cm=0)
        iota256 = sb.tile([P, 256], dt.int16, tag="iota256")
        nc.gpsimd.iota(iota256[:], pattern=[[1, 256]], base=0, channel_multiplier=0)
        iotaL = sb.tile([P, 16], dt.int16, tag="iotaL")
        nc.gpsimd.iota(iotaL[:], pattern=[[1, 16]], base=0, channel_multiplier=0)
        iotaf = sbd.tile([nch, 256], dt.float32, tag="iotaf")
        ioti = sbd.tile([nch, 256], dt.int32, tag="ioti")
        nc.gpsimd.iota(ioti[:], pattern=[[1, 256]], base=0, channel_multiplier=0)
        nc.vector.tensor_copy(iotaf[:], ioti[:])

        histos = sbd.tile([nch, 256], dt.float32, tag="histos")

        # ---------- Loop 1: histograms ----------
        for img in range(n_img):
            img8 = sb.tile([P, H * W * CH // P], dt.uint8, tag="img8")
            nc.sync.dma_start(out=img8[:], in_=imgs[img : img + 1, :])
            for c in range(CH):
                ch = img * CH + c
                x16 = sb.tile([P, F], dt.int16, tag="x16")
                nc.vector.tensor_copy(x16[:], img8[:, c::3])

                part = sb.tile([P, 256], dt.uint16, tag="part")
                for k in range(F // FH):
                    eq = sb.tile([P, 256 * FH], dt.uint8, tag="big")
                    # eq[p, b*FH + f] = (x16[p, k*FH + f] == b)
                    nc.vector.tensor_tensor(
                        out=eq[:],
                        in0=x16[:, k * FH : (k + 1) * FH]
                        .unsqueeze(1)
                        .to_broadcast([P, 256, FH]),
                        in1=iota256[:].unsqueeze(2).to_broadcast([P, 256, FH]),
                        op=Alu.is_equal,
                    )
                    pk = sb.tile([P, 256], dt.uint16, tag="pk")
                    with nc.allow_low_precision(
                        reason="integer counts <= 256 fit uint16 exactly"
                    ):
                        nc.vector.tensor_reduce(
                            out=pk[:],
                            in_=eq[:].rearrange("p (b f) -> p b f", f=FH),
                            axis=AX.X,
                            op=Alu.add,
                        )
                    if k == 0:
                        nc.vector.tensor_copy(part[:], pk[:])
                    else:
                        nc.vector.tensor_tensor(
                            out=part[:], in0=part[:], in1=pk[:], op=Alu.add
                        )
                # gather all 128 rows into one row, reduce with strided view
                row128 = sb.tile([1, P * 256], dt.uint16, tag="row128")
                nc.sync.dma_start(out=row128[:], in_=part[:])
                # row128[0, p*256 + b]; reduce over p via [1, 256(b), 128(p)]
                hrow = sb.tile([1, 256], dt.float32, tag="hrow")
                nc.vector.tensor_reduce(
                    out=hrow[:],
                    in_=row128[:].rearrange("o (pp b) -> o b pp", b=256),
                    axis=AX.X,
                    op=Alu.add,
                )
                nc.sync.dma_start(out=histos[ch : ch + 1, :], in_=hrow[:])

        # ---------- Batched LUT derivation on [nch, 256] ----------
        NC2 = nch
        ca = sbd.tile([NC2, 256], dt.float32, tag="ca")
        cb = sbd.tile([NC2, 256], dt.float32, tag="cb")
        src = histos
        for k in range(8):
            s = 1 << k
            dst = ca if (k % 2 == 0) else cb
            nc.vector.tensor_copy(dst[:, :s], src[:, :s])
            nc.vector.tensor_tensor(
                out=dst[:, s:256], in0=src[:, s:256], in1=src[:, : 256 - s],
                op=Alu.add,
            )
            src = dst
        cum = src  # cb
        t1 = ca

        nc.vector.tensor_scalar(
            out=t1[:], in0=cum[:], scalar1=float(NPX), scalar2=None, op0=Alu.is_lt
        )
        nc.vector.tensor_tensor(out=t1[:], in0=t1[:], in1=cum[:], op=Alu.mult)
        m2 = sbd.tile([NC2, 1], dt.float32, tag="m2")
        nc.vector.tensor_reduce(out=m2[:], in_=t1[:], axis=AX.X, op=Alu.max)

        stepf = sbd.tile([NC2, 1], dt.float32, tag="stepf")
        nc.vector.tensor_scalar(
            out=stepf[:], in0=m2[:], scalar1=1.0 / 255.0, scalar2=None, op0=Alu.mult
        )
        stepi = sbd.tile([NC2, 1], dt.int32, tag="stepi")
        nc.vector.tensor_copy(stepi[:], stepf[:])
        nc.vector.tensor_copy(stepf[:], stepi[:])
        se = sbd.tile([NC2, 1], dt.float32, tag="se")
        nc.vector.tensor_scalar(
            out=se[:], in0=stepf[:], scalar1=-255.0, scalar2=None, op0=Alu.mult
        )
        nc.vector.tensor_tensor(out=se[:], in0=m2[:], in1=se[:], op=Alu.add)
        scor = sbd.tile([NC2, 1], dt.float32, tag="scor")
        nc.vector.tensor_scalar(
            out=scor[:], in0=se[:], scalar1=0.0, scalar2=None, op0=Alu.is_lt
        )
        nc.vector.tensor_tensor(
            out=stepf[:], in0=stepf[:], in1=scor[:], op=Alu.subtract
        )
        nc.vector.tensor_scalar(
            out=scor[:], in0=se[:], scalar1=255.0, scalar2=None, op0=Alu.is_ge
        )
        nc.vector.tensor_tensor(out=stepf[:], in0=stepf[:], in1=scor[:], op=Alu.add)

        s_f = sbd.tile([NC2, 1], dt.float32, tag="s_f")
        nc.vector.tensor_scalar(
            out=s_f[:], in0=stepf[:], scalar1=1.0, scalar2=None, op0=Alu.max
        )
        halff = sbd.tile([NC2, 1], dt.float32, tag="halff")
        halfi = sbd.tile([NC2, 1], dt.int32, tag="halfi")
        nc.vector.tensor_scalar(
            out=halff[:], in0=s_f[:], scalar1=0.5, scalar2=-0.25,
            op0=Alu.mult, op1=Alu.add,
        )
        nc.vector.tensor_copy(halfi[:], halff[:])
        nc.vector.tensor_copy(halff[:], halfi[:])

        r0 = sbd.tile([NC2, 1], dt.float32, tag="r0")
        nc.vector.reciprocal(r0[:], s_f[:])
        tn = sbd.tile([NC2, 1], dt.float32, tag="tn")
        nc.vector.tensor_tensor(out=tn[:], in0=s_f[:], in1=r0[:], op=Alu.mult)
        nc.vector.tensor_scalar(
            out=tn[:], in0=tn[:], scalar1=-1.0, scalar2=2.0, op0=Alu.mult, op1=Alu.add
        )
        r1 = sbd.tile([NC2, 1], dt.float32, tag="r1")
        nc.vector.tensor_tensor(out=r1[:], in0=r0[:], in1=tn[:], op=Alu.mult)

        csp = sbd.tile([NC2, 256], dt.float32, tag="csp")
        nc.vector.memset(csp[:, :1], 0.0)
        nc.vector.tensor_copy(csp[:, 1:256], cum[:, :255])

        num = sbd.tile([NC2, 256], dt.float32, tag="num")
        nc.vector.tensor_scalar(
            out=num[:], in0=csp[:], scalar1=halff[:, :1], scalar2=r1[:, :1],
            op0=Alu.add, op1=Alu.mult,
        )
        q0i = sbd.tile([NC2, 256], dt.int32, tag="q0i")
        nc.vector.tensor_copy(q0i[:], num[:])
        q0 = sbd.tile([NC2, 256], dt.float32, tag="q0")
        nc.vector.tensor_copy(q0[:], q0i[:])

        e = sbd.tile([NC2, 256], dt.float32, tag="e")
        nc.vector.tensor_scalar(
            out=e[:], in0=q0[:], scalar1=s_f[:, :1], scalar2=None, op0=Alu.mult
        )
        nc.vector.tensor_tensor(out=e[:], in0=csp[:], in1=e[:], op=Alu.subtract)
        nc.vector.tensor_scalar(
            out=e[:], in0=e[:], scalar1=halff[:, :1], scalar2=None, op0=Alu.add
        )
        corr = sbd.tile([NC2, 256], dt.float32, tag="corr")
        nc.vector.tensor_scalar(
            out=corr[:], in0=e[:], scalar1=s_f[:, :1], scalar2=None, op0=Alu.is_ge
        )
        nc.vector.tensor_tensor(out=q0[:], in0=q0[:], in1=corr[:], op=Alu.add)
        nc.vector.tensor_scalar(
            out=corr[:], in0=e[:], scalar1=0.0, scalar2=None, op0=Alu.is_lt
        )
        nc.vector.tensor_tensor(out=q0[:], in0=q0[:], in1=corr[:], op=Alu.subtract)
        nc.vector.tensor_scalar(
            out=q0[:], in0=q0[:], scalar1=0.0, scalar2=255.0, op0=Alu.max, op1=Alu.min
        )

        m0 = sbd.tile([NC2, 1], dt.float32, tag="m0")
        nc.vector.tensor_scalar(
            out=m0[:], in0=stepf[:], scalar1=0.0, scalar2=None, op0=Alu.is_equal
        )
        lut = sbd.tile([NC2, 256], dt.float32, tag="lut")
        nc.vector.tensor_tensor(out=lut[:], in0=iotaf[:], in1=q0[:], op=Alu.subtract)
        nc.vector.tensor_scalar(
            out=lut[:], in0=lut[:], scalar1=m0[:, :1], scalar2=None, op0=Alu.mult
        )
        nc.vector.tensor_tensor(out=lut[:], in0=lut[:], in1=q0[:], op=Alu.add)
        lutb = sbd.tile([NC2, 256], dt.uint8, tag="lutb")
        nc.vector.tensor_copy(lutb[:], lut[:])
        if debug:
            nc.sync.dma_start(out=dbg[:, :], in_=lut[:])

        # ---------- Loop 2: apply ----------
        for img in range(n_img):
            img8b = sb.tile([P, H * W * CH // P], dt.uint8, tag="img8")
            nc.sync.dma_start(out=img8b[:], in_=imgs[img : img + 1, :])
            org = sb.tile([P, CH * F], dt.uint8, tag="org")
            for c in range(CH):
                ch = img * CH + c
                x16 = sb.tile([P, F], dt.int16, tag="x16")
                nc.vector.tensor_copy(x16[:], img8b[:, c::3])
                lo16 = sb.tile([P, F], dt.int16, tag="lo16")
                hi16 = sb.tile([P, F], dt.int16, tag="hi16")
                nc.vector.tensor_scalar(
                    out=lo16[:], in0=x16[:], scalar1=15, scalar2=None,
                    op0=Alu.bitwise_and,
                )
                nc.vector.tensor_scalar(
                    out=hi16[:], in0=x16[:], scalar1=4, scalar2=None,
                    op0=Alu.logical_shift_right,
                )
                # replicate this channel's lut row to all partitions, bf16
                T128 = sb.tile([P, 256], dt.uint8, tag="T128")
                nc.sync.dma_start(
                    out=T128[:],
                    in_=lutb[ch : ch + 1, :].unsqueeze(1).to_broadcast([1, P, 256]),
                )
                outb = sb.tile([P, F], dt.uint8, tag="outb")
                for k in range(F // FA):
                    sl = slice(k * FA, (k + 1) * FA)
                    # slabL chunk [P, 16l * FA] (l-major)
                    slabLc = sb.tile([P, 16 * FA], dt.uint8, tag="slabLc")
                    nc.vector.tensor_tensor(
                        out=slabLc[:],
                        in0=lo16[:, sl].unsqueeze(1).to_broadcast([P, 16, FA]),
                        in1=iotaL[:].unsqueeze(2).to_broadcast([P, 16, FA]),
                        op=Alu.is_equal,
                    )
                    slabHc = sb.tile([P, 16 * FA], dt.uint8, tag="slabHc")
                    nc.vector.tensor_tensor(
                        out=slabHc[:],
                        in0=hi16[:, sl].unsqueeze(1).to_broadcast([P, 16, FA]),
                        in1=iotaL[:].unsqueeze(2).to_broadcast([P, 16, FA]),
                        op=Alu.is_equal,
                    )
                    # prod[p, (h, f, l)] = slabLc[p, l*FA + f] * T128[p, 16h + l]
                    prod = sb.tile([P, 16 * FA * 16], dt.uint8, tag="big")
                    half = 8 * FA * 16
                    for hh in range(2):
                        nc.vector.tensor_tensor(
                            out=prod[:, hh * half : (hh + 1) * half],
                            in0=slabLc[:]
                            .rearrange("p (l f) -> p f l", l=16)
                            .unsqueeze(1)
                            .to_broadcast([P, 8, FA, 16]),
                            in1=T128[:, hh * 128 : (hh + 1) * 128]
                            .rearrange("p (h l) -> p h l", l=16)
                            .unsqueeze(2)
                            .to_broadcast([P, 8, FA, 16]),
                            op=Alu.mult,
                        )
                    # W[p, (h, f)] = sum_l prod
                    Wc = sb.tile([P, 16 * FA], dt.uint8, tag="Wc")
                    with nc.allow_low_precision(
                        reason="sums have exactly one nonzero bf16 term"
                    ):
                        nc.vector.tensor_reduce(
                        out=Wc[:],
                            in_=prod[:].rearrange(
                                "p (h f l) -> p (h f) l", l=16, f=FA
                            ),
                            axis=AX.X,
                            op=Alu.add,
                        )
                    # prod2[p, (f, h)] = slabHc * Wc (both (h, f) viewed as (f, h))
                    prod2 = sb.tile([P, FA * 16], dt.uint8, tag="prod2")
                    nc.vector.tensor_tensor(
                        out=prod2[:],
                        in0=slabHc[:].rearrange("p (h f) -> p f h", h=16),
                        in1=Wc[:].rearrange("p (h f) -> p f h", h=16),
                        op=Alu.mult,
                    )
                    with nc.allow_low_precision(
                        reason="sums have exactly one nonzero bf16 term"
                    ):
                        nc.vector.tensor_reduce(
                            out=outb[:, sl],
                            in_=prod2[:].rearrange("p (f h) -> p f h", h=16),
                            axis=AX.X,
                            op=Alu.add,
                        )
                # interleave into RGB layout (strided uint8 write)
                nc.vector.tensor_copy(org[:, c :: CH], outb[:])
            nc.sync.dma_start(out=out[img : img + 1, :], in_=org[:])

    nc.compile()
    return nc


def _get_fn():
    """Build the Bass program once and wrap it in a cached
    jax.jit(shard_map(bass_exec)) callable (the same lowering path
    run_bass_kernel_spmd takes under axon, minus its per-call re-trace,
    re-lower and zero-output upload)."""
    if "fn" in _cache:
        return _cache["fn"]

    import jax
    from jax.experimental.shard_map import shard_map
    from jax.sharding import Mesh, NamedSharding, PartitionSpec
    from concourse.bass2jax import (
        _bass_exec_p,
        install_neuronx_cc_hook,
        partition_id_tensor,
    )

    install_neuronx_cc_hook()

    n_img = CHUNK // N_CORES
    nc = build(n_img)
    out_avals = (jax.core.ShapedArray((n_img, H * W * CH), np.uint8),)

    def _body(imgs):
        outs = _bass_exec_p.bind(
            imgs,
            partition_id_tensor(),
            out_avals=out_avals,
            in_names=("imgs", "partition_id"),
            out_names=("out",),
            lowering_input_output_aliases=(),
            sim_require_finite=True,
            sim_require_nnan=True,
            nc=nc,
        )
        return outs[0]

    devices = jax.devices()[:N_CORES]
    mesh = Mesh(np.asarray(devices), ("core",))
    sharded = jax.jit(
        shard_map(
            _body,
            mesh=mesh,
            in_specs=(PartitionSpec("core"),),
            out_specs=PartitionSpec("core"),
            check_rep=False,
        )
    )
    sharding = NamedSharding(mesh, PartitionSpec("core"))
    _cache["fn"] = (sharded, sharding)
    return _cache["fn"]


def kernel(images: np.ndarray) -> np.ndarray:
    import jax

    fn, sharding = _get_fn()
    B = images.shape[0]

    futs = []
    for s in range(0, B, CHUNK):
        u8 = images[s : s + CHUNK].astype(np.uint8).reshape(CHUNK, -1)
        d = jax.device_put(u8, sharding)
        r = fn(d)
        try:
            r.copy_to_host_async()
        except Exception:
            pass
        futs.append(r)

    out = np.empty((B, H * W * CH), np.uint8)
    for i, r in enumerate(futs):
        out[i * CHUNK : (i + 1) * CHUNK] = np.asarray(r)
    return out.reshape(B, H, W, CH)
